# revision 21
# baseline (speedup 1.0000x reference)
"""BitNet transformer block on 8 Trainium2 NeuronCores (Bass/Tile SPMD).

v2: fold-balanced head-parallel attention (core i owns heads {2i,2i+1},
query blocks fold-paired; A2A to token-parallel), then pair-wise TP-2
MLP (cores {2j,2j+1} split INTER 4096/4096 over their 512 tokens) with
fp8e4m3 DoubleRow matmuls for q/k/v/gate/up (ternary weights are exact
in fp8), bf16 down-proj, and chunked pair ReduceScatter in bf16.
The o_proj residual x_mid is returned per-core and added on the host
during unshard assembly (as in v1).
"""

import sys

import numpy as np

try:
    import concourse.bass as bass  # noqa: F401
except Exception:  # pragma: no cover
    sys.path.insert(0, "/opt/trn_rl_repo")

import ml_dtypes
import concourse.bass as bass
import concourse.mybir as mybir
import concourse.tile as tile
from concourse import bacc
from concourse.bass_utils import run_bass_kernel_spmd

FP32 = mybir.dt.float32
BF16 = mybir.dt.bfloat16
FP8 = mybir.dt.float8e4
BF = ml_dtypes.bfloat16
F8 = ml_dtypes.float8_e4m3

ALPHA = 0.7
EPS = 1e-5
NH = 16          # query heads
NKV = 4          # kv heads
D = 128          # head dim
H = 2048         # hidden
I_TOT = 8192     # mlp intermediate
S = 2048         # sequence
NC = 8           # cores
P = 128
HT = H // P      # 16 hidden tiles
HT2 = HT // 2    # 8 hidden tile-pairs (fp8 DoubleRow)
B = S // P       # 16 token blocks
I_LOC = I_TOT // 2    # 4096 intermediate per core (TP-2)
IT = I_LOC // P       # 32 inter tiles per core
TOK = 256             # tokens owned per core (2 blocks)
PTOK = 512            # tokens owned per pair
DR = mybir.MatmulPerfMode.DoubleRow

_CACHE = {}


def _build_program():
    nc = bacc.Bacc("TRN2", target_bir_lowering=False, debug=False, num_devices=NC)
    AF = mybir.ActivationFunctionType
    ALU = mybir.AluOpType
    rg_all = [list(range(NC))]
    rg_pair = [[2 * j, 2 * j + 1] for j in range(NC // 2)]

    # ---------------- inputs ----------------
    def dram_in(name, shape, dt=FP32):
        return nc.dram_tensor(name, shape, dt, kind="ExternalInput")

    xT_f = dram_in("xT_f", [P, HT, S], FP8)           # fp8 x^T ALL tokens (ln1 only)
    xT_own = dram_in("xT_own", [P, HT, TOK])          # fp32 x^T own cols (residual)
    cos_f = dram_in("cos_f", [P, S], BF16)
    sin_f = dram_in("sin_f", [P, S], BF16)
    wq_in = dram_in("wq", [P, 2, HT2, 2, P], FP8)     # my 2 heads [p,f,b,i,m]
    wk_in = dram_in("wk", [P, HT2, 2, P], FP8)        # my kv head
    wv_in = dram_in("wv", [P, HT2, 2, P], FP8)
    wo_in = dram_in("wo", [HT, P, HT, P], FP8)
    wg_in = dram_in("wg", [IT, P, HT2, 2, P], FP8)    # [f,p,b,i,m]
    wu_in = dram_in("wu", [IT, P, HT2, 2, P], FP8)
    wd_in = dram_in("wd", [HT, P, IT // 2, 2, P], FP8)  # [fo,p,b,i,m]
    aq_in = dram_in("aq", [P, 2])
    ak_in = dram_in("ak", [P, 1])
    av_in = dram_in("av", [P, 1])
    ao_in = dram_in("ao", [P, HT])
    ag_in = dram_in("ag", [P, IT])
    au_in = dram_in("au", [P, IT])
    ad_in = dram_in("ad", [P, HT])
    rT_in = dram_in("rT", [P, P], BF16)               # rope rotate-half perm^T
    tril_in = dram_in("tril2", [P, TOK], BF16)        # [k, q] keep k<=q, both heads
    iden_in = dram_in("iden", [P, P], BF16)           # identity for PE transpose
    iden8_in = dram_in("iden8", [P, P], FP8)          # fp8 identity
    ones_f_in = dram_in("ones_f", [P, P])             # fp32 ones
    ones_b_in = dram_in("ones_b", [P, 1], BF16)       # bf16 ones column
    ones2_in = dram_in("ones2", [P, 2, 16], FP8)      # fp8 ones (DR lps lhsT)
    ones1_in = dram_in("ones1", [P, 16], FP8)         # fp8 ones (lps lhsT)
    invh_b_in = dram_in("invh_b", [P, 1], BF16)       # bf16 1/H column

    xmidT = nc.dram_tensor("xmidT", [P, HT, TOK], FP32, kind="ExternalOutput")
    outD = nc.dram_tensor("outD", [4, TOK, PTOK], BF16, kind="ExternalOutput")

    a2a_lo_in = nc.dram_tensor("a2a_lo_in", [NC, P, 2, P], FP8)
    a2a_lo_out = nc.dram_tensor("a2a_lo_out", [NC, P, 2, P], FP8)
    a2a_hi_in = nc.dram_tensor("a2a_hi_in", [NC, P, 2, P], FP8)
    a2a_hi_out = nc.dram_tensor("a2a_hi_out", [NC, P, 2, P], FP8)
    agx_in = nc.dram_tensor("agx_in", [P, HT, TOK], FP8)
    agx_out = nc.dram_tensor("agx_out", [2, P, HT, TOK], FP8)

    with tile.TileContext(nc) as tc:
        const = tc.alloc_tile_pool(name="const", bufs=1)
        ones_f = const.tile([P, P], FP32)
        ones_b = const.tile([P, 1], BF16)
        ones2 = const.tile([P, 2, 16], FP8)
        ones1 = const.tile([P, 16], FP8)
        invh_b = const.tile([P, 1], BF16)
        rT = const.tile([P, P], BF16)
        iden = const.tile([P, P], BF16)
        iden8 = const.tile([P, P], FP8)
        tril2 = const.tile([P, TOK], BF16)
        aq = const.tile([P, 2], FP32)
        ak = const.tile([P, 1], FP32)
        av = const.tile([P, 1], FP32)
        ao = const.tile([P, HT], FP32)
        ag = const.tile([P, IT], FP32)
        au = const.tile([P, IT], FP32)
        ad = const.tile([P, HT], FP32)
        for dst, src in [(ones_f, ones_f_in), (ones_b, ones_b_in),
                         (ones2, ones2_in), (ones1, ones1_in),
                         (invh_b, invh_b_in), (rT, rT_in),
                         (iden, iden_in), (iden8, iden8_in), (tril2, tril_in),
                         (aq, aq_in), (ak, ak_in), (av, av_in), (ao, ao_in),
                         (ag, ag_in), (au, au_in), (ad, ad_in)]:
            nc.sync.dma_start(dst[:], src[:])

        midpool = tc.alloc_tile_pool(name="midpool", bufs=1)
        x_mid = midpool.tile([P, HT, TOK], FP32)
        xopool = tc.alloc_tile_pool(name="xopool", bufs=1)
        xo = xopool.tile([P, HT, TOK], FP32)
        omypool = tc.alloc_tile_pool(name="omypool", bufs=1)
        o_my = omypool.tile([P, HT, TOK], FP8)      # post-A2A: 16 heads x my toks
        qkvpool = tc.alloc_tile_pool(name="qkvpool", bufs=1)
        q_my = qkvpool.tile([P, 2, S], BF16)         # my 2 heads, all tokens
        k_my = qkvpool.tile([P, B, P], BF16)         # my kv head [d, blk, tok]
        v_my = qkvpool.tile([P, B, P], FP8)          # my kv head [tok, blk, d]

        def rmsnorm_t(src3d, out3d, nt, psp, tmp, odt):
            """[P,HT,nt] -> rmsnorm, partition reduce via (1/H)-column matmul,
            then broadcast-matmul BEFORE reciprocal so DVE runs on 128 lanes."""
            ssq = psp.tile([1, 512], FP32, name="ssq")[:, :nt]
            for kt in range(HT):
                sqv = tmp.tile([P, 512], BF16, name="sqv")[:, :nt]
                eng = nc.gpsimd if kt % 3 == 2 else nc.vector
                eng.tensor_mul(sqv[:], src3d[:, kt, :], src3d[:, kt, :])
                nc.tensor.matmul(ssq[:], invh_b[:], sqv[:],
                                 start=(kt == 0), stop=(kt == HT - 1))
            ssb = tmp.tile([1, 512], FP32, name="ssb")[:, :nt]
            nc.scalar.activation(ssb[:], ssq[:], AF.Copy)
            msb = psp.tile([P, 512], FP32, name="msb")[:, :nt]
            nc.tensor.matmul(msb[:], ones_f[0:1, :], ssb[:], start=True, stop=True)
            rec = tmp.tile([P, 512], FP32, name="rec")[:, :nt]
            nc.vector.reciprocal_approx_fast(rec[:], msb[:])
            rsq = tmp.tile([P, 512], BF16, name="rsq")[:, :nt]
            nc.scalar.activation(rsq[:], rec[:], AF.Sqrt)
            for kt in range(HT):
                eng = nc.gpsimd if kt % 3 == 2 else nc.vector
                eng.tensor_mul(out3d[:, kt, :], src3d[:, kt, :], rsq[:])
            _ = odt

        # ====== phase 1: ln1 (all tokens, chunked) + q/k/v TP projections ======
        CH4 = 512
        with tc.tile_pool(name="xc_pool", bufs=2) as xcp, \
             tc.tile_pool(name="hc_pool", bufs=2) as hcp, \
             tc.tile_pool(name="p1sb", bufs=2) as p1sb, \
             tc.tile_pool(name="p1ps", bufs=1, space="PSUM") as p1ps, \
             tc.tile_pool(name="p2ps", bufs=2, space="PSUM") as p2ps, \
             tc.tile_pool(name="rot_ps", bufs=2, space="PSUM") as rot_ps, \
             tc.tile_pool(name="vt_ps", bufs=2, space="PSUM") as vt_ps, \
             tc.tile_pool(name="p2sb", bufs=2) as p2sb, \
             tc.tile_pool(name="cs_pool", bufs=2) as csp, \
             tc.tile_pool(name="wres", bufs=1) as wres:
            wq_sb = wres.tile([P, 2, HT2, 2, P], FP8)
            nc.sync.dma_start(wq_sb[:], wq_in[:])
            wk_sb = wres.tile([P, HT2, 2, P], FP8)
            nc.sync.dma_start(wk_sb[:], wk_in[:])
            wv_sb = wres.tile([P, HT2, 2, P], FP8)
            nc.sync.dma_start(wv_sb[:], wv_in[:])
            for c4 in range(4):
                tsl = slice(c4 * CH4, (c4 + 1) * CH4)
                xc = xcp.tile([P, HT, CH4], FP8, name="xc")
                nc.scalar.dma_start(xc[:], xT_f[:, :, tsl])
                cfc = csp.tile([P, CH4], BF16, name="cfc")
                nc.scalar.dma_start(cfc[:], cos_f[:, tsl])
                sfc = csp.tile([P, CH4], BF16, name="sfc")
                nc.scalar.dma_start(sfc[:], sin_f[:, tsl])
                # rmsnorm scale from fp8 x; applied per-token at PSUM evict
                ssq = p1ps.tile([1, CH4], FP32, name="ssq")
                for kt in range(HT):
                    sqv = p1sb.tile([P, CH4], BF16, name="sqv")
                    nc.vector.tensor_mul(sqv[:], xc[:, kt, :], xc[:, kt, :])
                    nc.tensor.matmul(ssq[:], invh_b[:], sqv[:],
                                     start=(kt == 0), stop=(kt == HT - 1))
                ssb = p1sb.tile([1, CH4], FP32, name="ssb")
                nc.scalar.activation(ssb[:], ssq[:], AF.Copy)
                msb = p1ps.tile([P, CH4], FP32, name="msb")
                nc.tensor.matmul(msb[:], ones_f[0:1, :], ssb[:],
                                 start=True, stop=True)
                rec = p1sb.tile([P, CH4], FP32, name="rec")
                nc.vector.reciprocal_approx_fast(rec[:], msb[:])
                rsq_bc = p1sb.tile([P, CH4], FP32, name="rsq_bc", tag="rsq_bc")
                nc.scalar.activation(rsq_bc[:], rec[:], AF.Sqrt)
                # q: my 2 heads (fp8 DoubleRow over hidden pairs)
                for f in range(2):
                    ps = p2ps.tile([P, CH4], FP32, name="pps")
                    for b in range(HT2):
                        nc.tensor.matmul(ps[:], wq_sb[:, f, b, :, :],
                                         xc[:, 2 * b:2 * b + 2, :],
                                         start=(b == 0), stop=(b == HT2 - 1),
                                         perf_mode=DR)
                    qs = p2sb.tile([P, CH4], BF16, name="qs")
                    nc.vector.scalar_tensor_tensor(qs[:], ps[:], aq[:, f:f + 1],
                                                   rsq_bc[:], ALU.mult, ALU.mult)
                    rot = rot_ps.tile([P, CH4], FP32, name="rot")
                    nc.tensor.matmul(rot[:], rT[:], qs[:], start=True, stop=True)
                    t1 = p2sb.tile([P, CH4], BF16, name="t1")
                    nc.vector.tensor_mul(t1[:], rot[:], sfc[:])
                    t2 = p2sb.tile([P, CH4], BF16, name="t2")
                    nc.vector.tensor_mul(t2[:], qs[:], cfc[:])
                    nc.gpsimd.tensor_add(q_my[:, f, tsl], t1[:], t2[:])
                # k: my kv head
                ps = p2ps.tile([P, CH4], FP32, name="pps")
                for b in range(HT2):
                    nc.tensor.matmul(ps[:], wk_sb[:, b, :, :],
                                     xc[:, 2 * b:2 * b + 2, :],
                                     start=(b == 0), stop=(b == HT2 - 1),
                                     perf_mode=DR)
                ks = p2sb.tile([P, CH4], BF16, name="qs")
                nc.vector.scalar_tensor_tensor(ks[:], ps[:], ak[:, 0:1],
                                               rsq_bc[:], ALU.mult, ALU.mult)
                rot = rot_ps.tile([P, CH4], FP32, name="rot")
                nc.tensor.matmul(rot[:], rT[:], ks[:], start=True, stop=True)
                t1 = p2sb.tile([P, CH4], BF16, name="t1")
                nc.vector.tensor_mul(t1[:], rot[:], sfc[:])
                t2 = p2sb.tile([P, CH4], BF16, name="t2")
                nc.vector.tensor_mul(t2[:], ks[:], cfc[:])
                nc.gpsimd.tensor_add(
                    k_my[:, 4 * c4:4 * c4 + 4, :].rearrange("p b t -> p (b t)"),
                    t1[:], t2[:])
                # v: my kv head, then PE-transpose to [tok, d] (fp8)
                ps = p2ps.tile([P, CH4], FP32, name="pps")
                for b in range(HT2):
                    nc.tensor.matmul(ps[:], wv_sb[:, b, :, :],
                                     xc[:, 2 * b:2 * b + 2, :],
                                     start=(b == 0), stop=(b == HT2 - 1),
                                     perf_mode=DR)
                vtv = p2sb.tile([P, CH4], BF16, name="vtv")
                nc.vector.scalar_tensor_tensor(vtv[:], ps[:], av[:, 0:1],
                                               rsq_bc[:], ALU.mult, ALU.mult)
                for j in range(4):
                    vtp = vt_ps.tile([P, P], BF16, name="vtp")
                    nc.tensor.transpose(vtp[:], vtv[:, j * P:(j + 1) * P], iden[:])
                    nc.vector.tensor_copy(v_my[:, 4 * c4 + j, :], vtp[:])

        # ============= phase 2: attention (triangle, paired heads) =============
        with tc.tile_pool(name="a_ps", bufs=3, space="PSUM") as a_ps, \
             tc.tile_pool(name="o_ps", bufs=2, space="PSUM") as o_ps, \
             tc.tile_pool(name="l_ps", bufs=2, space="PSUM") as l_ps, \
             tc.tile_pool(name="bc_ps", bufs=1, space="PSUM") as bc_ps, \
             tc.tile_pool(name="a_sb", bufs=3) as a_sb:
            for qb in range(B):
                r_dst = min(qb, 15 - qb)
                ops = o_ps.tile([P, TOK], FP32, name="ops")
                lps = l_ps.tile([16, TOK], FP32, name="lps")
                qv = q_my[:, :, qb * P:(qb + 1) * P]    # [P, 2, 128]
                npair = (qb + 1) // 2
                for g in range(npair + (qb + 1) % 2):
                    kb0 = 2 * g
                    first, last = kb0 == 0, kb0 + 2 > qb
                    if g < npair:   # full pair of key blocks (DoubleRow)
                        sps = a_ps.tile([P, 2, TOK], FP32, name="sps")
                        for j in range(2):
                            nc.tensor.matmul(sps[:, j, :], k_my[:, kb0 + j, :],
                                             qv, start=True, stop=True)
                        pm2 = a_sb.tile([P, 2, TOK], FP8, name="pm2")
                        if kb0 + 1 == qb:
                            nc.scalar.activation(pm2[:, 0, :], sps[:, 0, :],
                                                 AF.Exp)
                            pmd = a_sb.tile([P, TOK], BF16, name="pmd")
                            nc.scalar.activation(pmd[:], sps[:, 1, :], AF.Exp)
                            nc.vector.tensor_mul(pm2[:, 1, :], pmd[:], tril2[:])
                        else:
                            nc.scalar.activation(
                                pm2[:].rearrange("p a t -> p (a t)"),
                                sps[:].rearrange("p a t -> p (a t)"), AF.Exp)
                        nc.tensor.matmul(lps[:], ones2[:], pm2[:],
                                         start=first, stop=last, perf_mode=DR)
                        nc.tensor.matmul(ops[:], v_my[:, kb0:kb0 + 2, :], pm2[:],
                                         start=first, stop=last, perf_mode=DR)
                    else:           # leftover single (diagonal) block
                        sps = a_ps.tile([P, 2, TOK], FP32, name="sps")
                        nc.tensor.matmul(sps[:, 0, :], k_my[:, qb, :], qv,
                                         start=True, stop=True)
                        pmd = a_sb.tile([P, TOK], BF16, name="pmd")
                        nc.scalar.activation(pmd[:], sps[:, 0, :], AF.Exp)
                        pmf = a_sb.tile([P, TOK], FP8, name="pmf")
                        nc.vector.tensor_mul(pmf[:], pmd[:], tril2[:])
                        nc.tensor.matmul(lps[:], ones1[:], pmf[:],
                                         start=first, stop=True)
                        nc.tensor.matmul(ops[:], v_my[:, qb, :], pmf[:],
                                         start=first, stop=True)
                lsb = a_sb.tile([1, TOK], FP32, name="lsb")
                nc.scalar.activation(lsb[:], lps[0:1, :], AF.Copy)
                bca = bc_ps.tile([P, TOK], FP32, name="bca")
                nc.tensor.matmul(bca[:], ones_f[0:1, :], lsb[:], start=True, stop=True)
                linv = a_sb.tile([P, TOK], FP32, name="linv")
                nc.vector.reciprocal_approx_fast(linv[:], bca[:])
                osb = a_sb.tile([P, TOK], FP8, name="osb")
                nc.vector.tensor_mul(osb[:], ops[:], linv[:])
                dst = a2a_lo_in if qb < 8 else a2a_hi_in
                nc.sync.dma_start(
                    dst[r_dst][:],
                    osb[:].rearrange("p (h t) -> p h t", h=2))
                if qb == 7:
                    nc.gpsimd.collective_compute(
                        "AllToAll", ALU.bypass, ins=[a2a_lo_in[:]],
                        outs=[a2a_lo_out[:]], replica_groups=rg_all)
            nc.gpsimd.collective_compute(
                "AllToAll", ALU.bypass, ins=[a2a_hi_in[:]],
                outs=[a2a_hi_out[:]], replica_groups=rg_all)
        qkvpool.release()

        # ===== phase 3: o_proj + residual + ln2 (token halves) + pair-AG =====
        with tc.tile_pool(name="wo_pool", bufs=3) as wop, \
             tc.tile_pool(name="wo_res", bufs=1) as wores, \
             tc.tile_pool(name="p5ps", bufs=2, space="PSUM") as p5ps, \
             tc.tile_pool(name="p5sb", bufs=3) as p5sb:
            nc.sync.dma_start(xo[:], xT_own[:])
            wo_all = wores.tile([P, HT, HT, P], FP8)
            for f in range(HT):
                nc.scalar.dma_start(wo_all[:, f, :, :], wo_in[f])
            for j in range(NC):
                nc.sync.dma_start(o_my[:, 2 * j:2 * j + 2, 0:P], a2a_lo_out[j])
            for half in range(2):
                csl = slice(half * P, (half + 1) * P)
                if half == 1:
                    for j in range(NC):
                        nc.sync.dma_start(o_my[:, 2 * j:2 * j + 2, P:TOK],
                                          a2a_hi_out[j])
                for f in range(HT):
                    ps = p5ps.tile([P, P], FP32, name="ops5")
                    for kt in range(HT):
                        nc.tensor.matmul(ps[:], wo_all[:, f, kt, :], o_my[:, kt, csl],
                                         start=(kt == 0), stop=(kt == HT - 1))
                    nc.vector.scalar_tensor_tensor(
                        x_mid[:, f, csl], ps[:], ao[:, f:f + 1],
                        xo[:, f, csl], ALU.mult, ALU.add)
            h2h = p5sb.tile([P, HT, TOK], FP8, name="h2h", tag="h2h")
            rmsnorm_t(x_mid, h2h, TOK, p5ps, p5sb, FP8)
            nc.sync.dma_start(agx_in[:], h2h[:])
            nc.gpsimd.collective_compute(
                "AllGather", ALU.bypass, ins=[agx_in[:]],
                outs=[agx_out[:]], replica_groups=rg_pair)
            nc.sync.dma_start(xmidT[:], x_mid[:])
        omypool.release()
        xopool.release()

        # ========== phase 4: MLP (pair TP-2 over inter) + chunked RS ==========
        with tc.tile_pool(name="h2c_pool", bufs=1) as h2cp, \
             tc.tile_pool(name="m_pool", bufs=1) as mp, \
             tc.tile_pool(name="wgu_pool", bufs=4) as wgup, \
             tc.tile_pool(name="wd_pool", bufs=3) as wdp, \
             tc.tile_pool(name="p7ps", bufs=2, space="PSUM") as p7ps, \
             tc.tile_pool(name="p7dps", bufs=2, space="PSUM") as p7dps, \
             tc.tile_pool(name="p7sb", bufs=4) as p7sb:
            # pair token order: [2j's 256 | (2j+1)'s 256]
            h2c = h2cp.tile([P, HT, PTOK], FP8)
            for r in range(2):
                for kh in range(2):
                    eng = nc.sync if (2 * r + kh) % 2 == 0 else nc.scalar
                    eng.dma_start(h2c[:, 8 * kh:8 * (kh + 1), r * TOK:(r + 1) * TOK],
                                  agx_out[r][:, 8 * kh:8 * (kh + 1), :])
            m_all = mp.tile([P, IT, PTOK], FP8)
            for f in range(IT):
                wtg = wgup.tile([P, HT2, 2, P], FP8, name="wtg")
                nc.sync.dma_start(wtg[:], wg_in[f])
                gps = p7ps.tile([P, PTOK], FP32, name="gps")
                for b in range(HT2):
                    nc.tensor.matmul(gps[:], wtg[:, b, :, :],
                                     h2c[:, 2 * b:2 * b + 2, :],
                                     start=(b == 0), stop=(b == HT2 - 1),
                                     perf_mode=DR)
                wtu = wgup.tile([P, HT2, 2, P], FP8, name="wtu")
                nc.sync.dma_start(wtu[:], wu_in[f])
                ups = p7ps.tile([P, PTOK], FP32, name="ups")
                for b in range(HT2):
                    nc.tensor.matmul(ups[:], wtu[:, b, :, :],
                                     h2c[:, 2 * b:2 * b + 2, :],
                                     start=(b == 0), stop=(b == HT2 - 1),
                                     perf_mode=DR)
                gr = p7sb.tile([P, PTOK], BF16, name="gr")
                nc.vector.tensor_scalar(gr[:], gps[:], ag[:, f:f + 1], 0.0,
                                        ALU.mult, ALU.max)
                g2 = p7sb.tile([P, PTOK], BF16, name="g2")
                nc.gpsimd.tensor_mul(g2[:], gr[:], gr[:])
                nc.vector.scalar_tensor_tensor(m_all[:, f, :], ups[:],
                                               au[:, f:f + 1], g2[:],
                                               ALU.mult, ALU.mult)
            # down proj (bf16) + chunked pair-RS (4 chunks of 4 f-tiles)
            for c in range(4):
                rs_in = nc.dram_tensor(f"rs_in_{c}", [4 * P, PTOK], BF16)
                rs_iv = rs_in[:].rearrange("(f p) t -> f p t", p=P)
                rs_out = nc.dram_tensor(f"rs_out_{c}", [2 * P, PTOK], BF16)
                for fi in range(4):
                    fo = 4 * c + fi
                    wtd = wdp.tile([P, IT // 2, 2, P], FP8, name="wtd")
                    nc.sync.dma_start(wtd[:], wd_in[fo])
                    dps = p7dps.tile([P, PTOK], FP32, name="dps")
                    for b in range(IT // 2):
                        nc.tensor.matmul(dps[:], wtd[:, b, :, :],
                                         m_all[:, 2 * b:2 * b + 2, :],
                                         start=(b == 0), stop=(b == IT // 2 - 1),
                                         perf_mode=DR)
                    dn = p7sb.tile([P, PTOK], BF16, name="dn")
                    nc.scalar.activation(dn[:], dps[:], AF.Copy,
                                         scale=ad[:, fo:fo + 1])
                    nc.sync.dma_start(rs_iv[fo % 4], dn[:])
                nc.gpsimd.collective_compute(
                    "ReduceScatter", ALU.add, ins=[rs_in[:]],
                    outs=[rs_out[:]], replica_groups=rg_pair)
                nc.sync.dma_start(outD[c], rs_out[:])
            _ = wop
        midpool.release()
        const.release()

    nc.finalize()
    return nc


def _ternary(w, fold_row=None):
    """Quantize [O, Hin] fp32 -> (ternary fp32 {-1,0,1}, absmean [O])."""
    w = np.asarray(w, dtype=np.float32)
    am = np.mean(np.abs(w), axis=1)
    t = np.sign(w) * (np.abs(w) > ALPHA * am[:, None]).astype(np.float32)
    if fold_row is not None:
        t = t * fold_row[None, :]
    return t, am


def _wlhsT(tern, n_f):
    """ternary [O, Hin] -> bf16 lhsT layout [f, p, kt, c]."""
    o, hin = tern.shape
    kt = hin // P
    assert n_f * P == o
    wT = np.ascontiguousarray(tern.T)  # [Hin, O]
    return np.ascontiguousarray(
        wT.reshape(kt, P, n_f, P).transpose(2, 1, 0, 3)).astype(BF)


def _wlhsT_dr(tern, n_f):
    """ternary [O, Hin] -> fp8 DoubleRow lhsT layout [p, f, b, i, m]:
    w[p, f, b, i, m] = ternT[128*(2b+i)+p, 128*f+m]."""
    o, hin = tern.shape
    b2 = hin // (2 * P)
    assert n_f * P == o
    wT = np.ascontiguousarray(tern.T)  # [Hin, O]
    return np.ascontiguousarray(
        wT.reshape(b2, 2, P, n_f, P).transpose(2, 3, 0, 1, 4)).astype(F8)


def _wd_layout(td_slice):
    """[H, I_loc] -> fp8 DoubleRow [fo, p, b, i, m]: wd[fo, p, b, i, m] =
    td_slice[128*fo+m, 128*(2b+i)+p]."""
    hin, iloc = td_slice.shape
    assert hin == H and iloc == I_LOC
    wT = np.ascontiguousarray(td_slice.T)  # [I_loc, H]
    return np.ascontiguousarray(
        wT.reshape(IT // 2, 2, P, HT, P).transpose(3, 2, 0, 1, 4)).astype(F8)


def _scale_tiles(a):
    """[O] -> [P, O//P] with column f = features f*128..f*128+127."""
    return np.ascontiguousarray(a.reshape(-1, P).T).astype(np.float32)


def _pcol(x2d):
    """[K, T] -> [P, K//P, T] (partition-major for direct DMA)."""
    k, t = x2d.shape
    return np.ascontiguousarray(
        x2d.reshape(k // P, P, t).transpose(1, 0, 2)).astype(np.float32)


def kernel(x, cos, sin, wq, wk, wv, wo, wg, wu, wd, ln1_w, ln2_w):
    x = np.asarray(x, dtype=np.float32)
    b, s, hdim = x.shape
    assert (b, s, hdim) == (1, S, H)

    if "nc" not in _CACHE:
        _CACHE["nc"] = _build_program()
    nc = _CACHE["nc"]

    ln1 = np.asarray(ln1_w, dtype=np.float32)
    ln2 = np.asarray(ln2_w, dtype=np.float32)

    tq, amq = _ternary(wq, fold_row=ln1)
    tk, amk = _ternary(wk, fold_row=ln1)
    tv, amv = _ternary(wv, fold_row=ln1)
    to, amo = _ternary(wo)
    tg, amg = _ternary(wg, fold_row=ln2)
    tu, amu = _ternary(wu, fold_row=ln2)
    td, amd = _ternary(wd)

    wq_h = _wlhsT_dr(tq, NH)          # [P, 16, 8, 2, P]
    wk_h = _wlhsT_dr(tk, NKV)         # [P, 4, 8, 2, P]
    wv_h = _wlhsT_dr(tv, NKV)
    wo_h = _wlhsT(to, HT).astype(F8)  # [16, P, 16, P] fp8

    aq_h = _scale_tiles(amq / np.sqrt(np.float32(D)))
    ak_h = _scale_tiles(amk)
    av_h = _scale_tiles(amv)
    ao_h = _scale_tiles(amo)
    ag_h = _scale_tiles(amg)          # [P, 64]
    au_h = _scale_tiles(amu)
    ad_h = _scale_tiles(amd)          # [P, 16]

    x2 = x[0]
    xT = np.ascontiguousarray(x2.T)
    xT_f = _pcol(xT)
    cosT = np.ascontiguousarray(np.asarray(cos, np.float32)[0, 0].T).astype(BF)
    sinT = np.ascontiguousarray(np.asarray(sin, np.float32)[0, 0].T).astype(BF)

    R = np.zeros((P, P), np.float32)
    for m in range(64):
        R[m, m + 64] = -1.0
        R[m + 64, m] = 1.0
    rT_h = np.ascontiguousarray(R.T).astype(BF)
    ones_f = np.ones((P, P), np.float32)
    ones_b = np.ones((P, 1), np.float32).astype(BF)
    invh_b = np.full((P, 1), 1.0 / H, np.float32).astype(BF)
    ones2_h = np.ones((P, 2, 16), np.float32).astype(F8)
    ones1_h = np.ones((P, 16), np.float32).astype(F8)
    triu = np.triu(np.ones((P, P), np.float32))
    tril2_h = np.ascontiguousarray(np.concatenate([triu, triu], axis=1)).astype(BF)
    iden_h = np.eye(P, dtype=np.float32).astype(BF)

    in_maps = []
    for i in range(NC):
        blo, bhi = i, 15 - i
        own_cols = np.r_[blo * P:(blo + 1) * P, bhi * P:(bhi + 1) * P]
        kvh = i // 2
        par = i % 2
        isl = slice(par * IT, (par + 1) * IT)       # inter tile slice (TP-2)
        irow = slice(par * I_LOC, (par + 1) * I_LOC)
        in_maps.append({
            "xT_f": xT_f.astype(F8),
            "xT_own": _pcol(xT[:, own_cols]),
            "cos_f": cosT, "sin_f": sinT,
            "wq": np.ascontiguousarray(wq_h[:, 2 * i:2 * i + 2]),
            "wk": np.ascontiguousarray(wk_h[:, kvh]),
            "wv": np.ascontiguousarray(wv_h[:, kvh]),
            "wo": wo_h,
            "wg": np.ascontiguousarray(
                _wlhsT_dr(tg[irow], IT).transpose(1, 0, 2, 3, 4)),
            "wu": np.ascontiguousarray(
                _wlhsT_dr(tu[irow], IT).transpose(1, 0, 2, 3, 4)),
            "wd": _wd_layout(td[:, irow]),
            "aq": np.ascontiguousarray(aq_h[:, 2 * i:2 * i + 2]),
            "ak": np.ascontiguousarray(ak_h[:, kvh:kvh + 1]),
            "av": np.ascontiguousarray(av_h[:, kvh:kvh + 1]),
            "ao": ao_h,
            "ag": np.ascontiguousarray(ag_h[:, isl]),
            "au": np.ascontiguousarray(au_h[:, isl]),
            "ad": ad_h,
            "rT": rT_h, "tril2": tril2_h, "iden": iden_h,
            "iden8": iden_h.astype(F8),
            "ones_f": ones_f, "ones_b": ones_b, "invh_b": invh_b,
            "ones2": ones2_h, "ones1": ones1_h,
        })

    res = run_bass_kernel_spmd(nc, in_maps, list(range(NC)))
    _CACHE["last_result"] = res

    # ---- host-side unshard: xmid residual + pair-RS output assembly ----
    out_T = np.zeros((H, S), np.float64)
    for i in range(NC):
        blo, bhi = i, 15 - i
        xm = res.results[i]["xmidT"].astype(np.float64)      # [P, HT, 256]
        xm = xm.transpose(1, 0, 2).reshape(H, TOK)
        out_T[:, blo * P:(blo + 1) * P] += xm[:, 0:P]
        out_T[:, bhi * P:(bhi + 1) * P] += xm[:, P:TOK]
    for j in range(NC // 2):
        # pair token order: [core 2j's 256 | core 2j+1's 256]
        tok_cols = np.r_[(2 * j) * P:(2 * j + 1) * P,
                         (15 - 2 * j) * P:(16 - 2 * j) * P,
                         (2 * j + 1) * P:(2 * j + 2) * P,
                         (14 - 2 * j) * P:(15 - 2 * j) * P]
        for par in range(2):
            od = res.results[2 * j + par]["outD"].astype(np.float64)  # [4,256,512]
            for c in range(4):
                rows = slice(512 * c + 256 * par, 512 * c + 256 * (par + 1))
                out_T[rows][:, tok_cols] += od[c]
    return np.ascontiguousarray(out_T.T).reshape(1, S, H).astype(np.float32)


if __name__ == "__main__":
    nc = _build_program()
    print("build OK; instructions:",
          sum(len(b.instructions) for f in nc.m.functions for b in f.blocks))


# revision 23
# speedup vs baseline: 1.0996x; 1.0996x over previous
"""BitNet transformer block on 8 Trainium2 NeuronCores (Bass/Tile SPMD).

v2: fold-balanced head-parallel attention (core i owns heads {2i,2i+1},
query blocks fold-paired; A2A to token-parallel), then pair-wise TP-2
MLP (cores {2j,2j+1} split INTER 4096/4096 over their 512 tokens) with
fp8e4m3 DoubleRow matmuls for q/k/v/gate/up (ternary weights are exact
in fp8), bf16 down-proj, and chunked pair ReduceScatter in bf16.
The o_proj residual x_mid is returned per-core and added on the host
during unshard assembly (as in v1).
"""

import sys

import numpy as np

try:
    import concourse.bass as bass  # noqa: F401
except Exception:  # pragma: no cover
    sys.path.insert(0, "/opt/trn_rl_repo")

import ml_dtypes
import concourse.bass as bass
import concourse.mybir as mybir
import concourse.tile as tile
from concourse import bacc
from concourse.bass_utils import run_bass_kernel_spmd

FP32 = mybir.dt.float32
BF16 = mybir.dt.bfloat16
FP8 = mybir.dt.float8e4
BF = ml_dtypes.bfloat16
F8 = ml_dtypes.float8_e4m3

ALPHA = 0.7
EPS = 1e-5
NH = 16          # query heads
NKV = 4          # kv heads
D = 128          # head dim
H = 2048         # hidden
I_TOT = 8192     # mlp intermediate
S = 2048         # sequence
NC = 8           # cores
P = 128
HT = H // P      # 16 hidden tiles
HT2 = HT // 2    # 8 hidden tile-pairs (fp8 DoubleRow)
B = S // P       # 16 token blocks
I_LOC = I_TOT // 2    # 4096 intermediate per core (TP-2)
IT = I_LOC // P       # 32 inter tiles per core
TOK = 256             # tokens owned per core (2 blocks)
PTOK = 512            # tokens owned per pair
DR = mybir.MatmulPerfMode.DoubleRow

_CACHE = {}


def _build_program():
    nc = bacc.Bacc("TRN2", target_bir_lowering=False, debug=False, num_devices=NC)
    AF = mybir.ActivationFunctionType
    ALU = mybir.AluOpType
    rg_all = [list(range(NC))]
    rg_pair = [[2 * j, 2 * j + 1] for j in range(NC // 2)]

    # ---------------- inputs ----------------
    def dram_in(name, shape, dt=FP32):
        return nc.dram_tensor(name, shape, dt, kind="ExternalInput")

    xT_f = dram_in("xT_f", [P, HT, S], FP8)           # fp8 x^T ALL tokens (ln1 only)
    xT_own = dram_in("xT_own", [P, HT, TOK])          # fp32 x^T own cols (residual)
    cos_f = dram_in("cos_f", [P, S], BF16)
    sin_f = dram_in("sin_f", [P, S], BF16)
    wq_in = dram_in("wq", [P, 2, HT2, 2, P], FP8)     # my 2 heads [p,f,b,i,m]
    wk_in = dram_in("wk", [P, HT2, 2, P], FP8)        # my kv head
    wv_in = dram_in("wv", [P, HT2, 2, P], FP8)
    wo_in = dram_in("wo", [HT, P, HT, P], FP8)
    wg_in = dram_in("wg", [IT // 8, P, 8, HT2, 2, P], FP8)  # [g,p,f8,b,i,m]
    wu_in = dram_in("wu", [IT // 8, P, 8, HT2, 2, P], FP8)
    wd_in = dram_in("wd", [HT, P, IT // 2, 2, P], FP8)  # [fo,p,b,i,m]
    aq_in = dram_in("aq", [P, 2])
    ak_in = dram_in("ak", [P, 1])
    av_in = dram_in("av", [P, 1])
    ao_in = dram_in("ao", [P, HT])
    ag_in = dram_in("ag", [P, IT])
    au_in = dram_in("au", [P, IT])
    ad_in = dram_in("ad", [P, HT])
    rT_in = dram_in("rT", [P, P], BF16)               # rope rotate-half perm^T
    tril_in = dram_in("tril2", [P, TOK], BF16)        # [k, q] keep k<=q, both heads
    iden_in = dram_in("iden", [P, P], BF16)           # identity for PE transpose
    iden8_in = dram_in("iden8", [P, P], FP8)          # fp8 identity
    ones_f_in = dram_in("ones_f", [P, P])             # fp32 ones
    ones_b_in = dram_in("ones_b", [P, 1], BF16)       # bf16 ones column
    ones2_in = dram_in("ones2", [P, 2, 16], FP8)      # fp8 ones (DR lps lhsT)
    ones1_in = dram_in("ones1", [P, 16], FP8)         # fp8 ones (lps lhsT)
    invh_b_in = dram_in("invh_b", [P, 1], BF16)       # bf16 1/H column
    invh2_in = dram_in("invh2", [P, 2, 16], FP8)      # fp8 2^-9 (DR ssq lhsT)

    xmidT = nc.dram_tensor("xmidT", [P, HT, TOK], FP32, kind="ExternalOutput")
    outD = nc.dram_tensor("outD", [4, TOK, PTOK], BF16, kind="ExternalOutput")

    a2a_lo_in = nc.dram_tensor("a2a_lo_in", [NC, P, 2, P], FP8)
    a2a_lo_out = nc.dram_tensor("a2a_lo_out", [NC, P, 2, P], FP8)
    a2a_hi_in = nc.dram_tensor("a2a_hi_in", [NC, P, 2, P], FP8)
    a2a_hi_out = nc.dram_tensor("a2a_hi_out", [NC, P, 2, P], FP8)
    agx_in = nc.dram_tensor("agx_in", [P, HT, TOK], FP8)
    agx_out = nc.dram_tensor("agx_out", [2, P, HT, TOK], FP8)

    with tile.TileContext(nc) as tc:
        const = tc.alloc_tile_pool(name="const", bufs=1)
        ones_f = const.tile([P, P], FP32)
        ones_b = const.tile([P, 1], BF16)
        ones2 = const.tile([P, 2, 16], FP8)
        ones1 = const.tile([P, 16], FP8)
        invh_b = const.tile([P, 1], BF16)
        invh2 = const.tile([P, 2, 16], FP8)
        rT = const.tile([P, P], BF16)
        iden = const.tile([P, P], BF16)
        iden8 = const.tile([P, P], FP8)
        tril2 = const.tile([P, TOK], BF16)
        aq = const.tile([P, 2], FP32)
        ak = const.tile([P, 1], FP32)
        av = const.tile([P, 1], FP32)
        ao = const.tile([P, HT], FP32)
        ag = const.tile([P, IT], FP32)
        au = const.tile([P, IT], FP32)
        ad = const.tile([P, HT], FP32)
        for dst, src in [(ones_f, ones_f_in), (ones_b, ones_b_in),
                         (ones2, ones2_in), (ones1, ones1_in),
                         (invh_b, invh_b_in), (invh2, invh2_in), (rT, rT_in),
                         (iden, iden_in), (iden8, iden8_in), (tril2, tril_in),
                         (aq, aq_in), (ak, ak_in), (av, av_in), (ao, ao_in),
                         (ag, ag_in), (au, au_in), (ad, ad_in)]:
            nc.sync.dma_start(dst[:], src[:])

        midpool = tc.alloc_tile_pool(name="midpool", bufs=1)
        x_mid = midpool.tile([P, HT, TOK], FP32)
        xopool = tc.alloc_tile_pool(name="xopool", bufs=1)
        xo = xopool.tile([P, HT, TOK], FP32)
        omypool = tc.alloc_tile_pool(name="omypool", bufs=1)
        o_my = omypool.tile([P, HT, TOK], FP8)      # post-A2A: 16 heads x my toks
        qkvpool = tc.alloc_tile_pool(name="qkvpool", bufs=1)
        q_my = qkvpool.tile([P, 2, S], BF16)         # my 2 heads, all tokens
        k_my = qkvpool.tile([P, B, P], BF16)         # my kv head [d, blk, tok]
        v_my = qkvpool.tile([P, B, P], FP8)          # my kv head [tok, blk, d]

        def rmsnorm_t(src3d, out3d, nt, psp, tmp, odt):
            """[P,HT,nt] -> rmsnorm, partition reduce via (1/H)-column matmul,
            then broadcast-matmul BEFORE reciprocal so DVE runs on 128 lanes."""
            ssq = psp.tile([1, 512], FP32, name="ssq")[:, :nt]
            for kt in range(HT):
                sqv = tmp.tile([P, 512], BF16, name="sqv")[:, :nt]
                eng = nc.gpsimd if kt % 3 == 2 else nc.vector
                eng.tensor_mul(sqv[:], src3d[:, kt, :], src3d[:, kt, :])
                nc.tensor.matmul(ssq[:], invh_b[:], sqv[:],
                                 start=(kt == 0), stop=(kt == HT - 1))
            ssb = tmp.tile([1, 512], FP32, name="ssb")[:, :nt]
            nc.scalar.activation(ssb[:], ssq[:], AF.Copy)
            msb = psp.tile([P, 512], FP32, name="msb")[:, :nt]
            nc.tensor.matmul(msb[:], ones_f[0:1, :], ssb[:], start=True, stop=True)
            rec = tmp.tile([P, 512], FP32, name="rec")[:, :nt]
            nc.vector.reciprocal_approx_fast(rec[:], msb[:])
            rsq = tmp.tile([P, 512], BF16, name="rsq")[:, :nt]
            nc.scalar.activation(rsq[:], rec[:], AF.Sqrt)
            for kt in range(HT):
                eng = nc.gpsimd if kt % 3 == 2 else nc.vector
                eng.tensor_mul(out3d[:, kt, :], src3d[:, kt, :], rsq[:])
            _ = odt

        # ====== phase 1: ln1 (all tokens, chunked) + q/k/v TP projections ======
        CH4 = 512
        with tc.tile_pool(name="xc_pool", bufs=2) as xcp, \
             tc.tile_pool(name="hc_pool", bufs=2) as hcp, \
             tc.tile_pool(name="p1sb", bufs=2) as p1sb, \
             tc.tile_pool(name="p1ps", bufs=1, space="PSUM") as p1ps, \
             tc.tile_pool(name="p2ps", bufs=2, space="PSUM") as p2ps, \
             tc.tile_pool(name="rot_ps", bufs=2, space="PSUM") as rot_ps, \
             tc.tile_pool(name="vt_ps", bufs=2, space="PSUM") as vt_ps, \
             tc.tile_pool(name="p2sb", bufs=2) as p2sb, \
             tc.tile_pool(name="cs_pool", bufs=2) as csp, \
             tc.tile_pool(name="wres", bufs=1) as wres:
            wq_sb = wres.tile([P, 2, HT2, 2, P], FP8)
            nc.sync.dma_start(wq_sb[:], wq_in[:])
            wk_sb = wres.tile([P, HT2, 2, P], FP8)
            nc.sync.dma_start(wk_sb[:], wk_in[:])
            wv_sb = wres.tile([P, HT2, 2, P], FP8)
            nc.sync.dma_start(wv_sb[:], wv_in[:])
            for c4 in range(4):
                tsl = slice(c4 * CH4, (c4 + 1) * CH4)
                xc = xcp.tile([P, HT, CH4], FP8, name="xc")
                nc.scalar.dma_start(xc[:], xT_f[:, :, tsl])
                cfc = csp.tile([P, CH4], BF16, name="cfc")
                nc.scalar.dma_start(cfc[:], cos_f[:, tsl])
                sfc = csp.tile([P, CH4], BF16, name="sfc")
                nc.scalar.dma_start(sfc[:], sin_f[:, tsl])
                # rmsnorm scale from fp8 x; applied per-token at PSUM evict
                ssq = p1ps.tile([16, CH4], FP32, name="ssq")
                for b in range(HT2):
                    sq2 = p1sb.tile([P, 2, CH4], FP8, name="sq2")
                    for j in range(2):
                        nc.vector.tensor_mul(sq2[:, j, :], xc[:, 2 * b + j, :],
                                             xc[:, 2 * b + j, :])
                    nc.tensor.matmul(ssq[:], invh2[:], sq2[:],
                                     start=(b == 0), stop=(b == HT2 - 1),
                                     perf_mode=DR)
                ssb = p1sb.tile([1, CH4], FP32, name="ssb")
                nc.scalar.activation(ssb[:], ssq[0:1, :], AF.Copy)
                msb = p1ps.tile([P, CH4], FP32, name="msb")
                nc.tensor.matmul(msb[:], ones_f[0:1, :], ssb[:],
                                 start=True, stop=True)
                rec = p1sb.tile([P, CH4], FP32, name="rec")
                nc.vector.reciprocal_approx_fast(rec[:], msb[:])
                rsq_bc = p1sb.tile([P, CH4], FP32, name="rsq_bc", tag="rsq_bc")
                nc.scalar.activation(rsq_bc[:], rec[:], AF.Sqrt, scale=4.0)
                # q: my 2 heads (fp8 DoubleRow over hidden pairs)
                for f in range(2):
                    ps = p2ps.tile([P, CH4], FP32, name="pps")
                    for b in range(HT2):
                        nc.tensor.matmul(ps[:], wq_sb[:, f, b, :, :],
                                         xc[:, 2 * b:2 * b + 2, :],
                                         start=(b == 0), stop=(b == HT2 - 1),
                                         perf_mode=DR)
                    qs = p2sb.tile([P, CH4], BF16, name="qs")
                    nc.vector.scalar_tensor_tensor(qs[:], ps[:], aq[:, f:f + 1],
                                                   rsq_bc[:], ALU.mult, ALU.mult)
                    rot = rot_ps.tile([P, CH4], FP32, name="rot")
                    nc.tensor.matmul(rot[:], rT[:], qs[:], start=True, stop=True)
                    t1 = p2sb.tile([P, CH4], BF16, name="t1")
                    nc.vector.tensor_mul(t1[:], rot[:], sfc[:])
                    t2 = p2sb.tile([P, CH4], BF16, name="t2")
                    nc.vector.tensor_mul(t2[:], qs[:], cfc[:])
                    nc.gpsimd.tensor_add(q_my[:, f, tsl], t1[:], t2[:])
                # k: my kv head
                ps = p2ps.tile([P, CH4], FP32, name="pps")
                for b in range(HT2):
                    nc.tensor.matmul(ps[:], wk_sb[:, b, :, :],
                                     xc[:, 2 * b:2 * b + 2, :],
                                     start=(b == 0), stop=(b == HT2 - 1),
                                     perf_mode=DR)
                ks = p2sb.tile([P, CH4], BF16, name="qs")
                nc.vector.scalar_tensor_tensor(ks[:], ps[:], ak[:, 0:1],
                                               rsq_bc[:], ALU.mult, ALU.mult)
                rot = rot_ps.tile([P, CH4], FP32, name="rot")
                nc.tensor.matmul(rot[:], rT[:], ks[:], start=True, stop=True)
                t1 = p2sb.tile([P, CH4], BF16, name="t1")
                nc.vector.tensor_mul(t1[:], rot[:], sfc[:])
                t2 = p2sb.tile([P, CH4], BF16, name="t2")
                nc.vector.tensor_mul(t2[:], ks[:], cfc[:])
                nc.gpsimd.tensor_add(
                    k_my[:, 4 * c4:4 * c4 + 4, :].rearrange("p b t -> p (b t)"),
                    t1[:], t2[:])
                # v: my kv head, then PE-transpose to [tok, d] (fp8)
                ps = p2ps.tile([P, CH4], FP32, name="pps")
                for b in range(HT2):
                    nc.tensor.matmul(ps[:], wv_sb[:, b, :, :],
                                     xc[:, 2 * b:2 * b + 2, :],
                                     start=(b == 0), stop=(b == HT2 - 1),
                                     perf_mode=DR)
                vtv = p2sb.tile([P, CH4], BF16, name="vtv")
                nc.vector.scalar_tensor_tensor(vtv[:], ps[:], av[:, 0:1],
                                               rsq_bc[:], ALU.mult, ALU.mult)
                for j in range(4):
                    vtp = vt_ps.tile([P, P], BF16, name="vtp")
                    nc.tensor.transpose(vtp[:], vtv[:, j * P:(j + 1) * P], iden[:])
                    nc.vector.tensor_copy(v_my[:, 4 * c4 + j, :], vtp[:])

        # ============= phase 2: attention (triangle, paired heads) =============
        with tc.tile_pool(name="a_ps", bufs=3, space="PSUM") as a_ps, \
             tc.tile_pool(name="o_ps", bufs=2, space="PSUM") as o_ps, \
             tc.tile_pool(name="l_ps", bufs=2, space="PSUM") as l_ps, \
             tc.tile_pool(name="bc_ps", bufs=1, space="PSUM") as bc_ps, \
             tc.tile_pool(name="a_sb", bufs=3) as a_sb:
            for qb in range(B):
                r_dst = min(qb, 15 - qb)
                ops = o_ps.tile([P, TOK], FP32, name="ops")
                lps = l_ps.tile([16, TOK], FP32, name="lps")
                qv = q_my[:, :, qb * P:(qb + 1) * P]    # [P, 2, 128]
                npair = (qb + 1) // 2
                for g in range(npair + (qb + 1) % 2):
                    kb0 = 2 * g
                    first, last = kb0 == 0, kb0 + 2 > qb
                    if g < npair:   # full pair of key blocks (DoubleRow)
                        sps = a_ps.tile([P, 2, TOK], FP32, name="sps")
                        for j in range(2):
                            nc.tensor.matmul(sps[:, j, :], k_my[:, kb0 + j, :],
                                             qv, start=True, stop=True)
                        pm2 = a_sb.tile([P, 2, TOK], FP8, name="pm2")
                        if kb0 + 1 == qb:
                            nc.scalar.activation(pm2[:, 0, :], sps[:, 0, :],
                                                 AF.Exp)
                            pmd = a_sb.tile([P, TOK], BF16, name="pmd")
                            nc.scalar.activation(pmd[:], sps[:, 1, :], AF.Exp)
                            nc.vector.tensor_mul(pm2[:, 1, :], pmd[:], tril2[:])
                        else:
                            nc.scalar.activation(
                                pm2[:].rearrange("p a t -> p (a t)"),
                                sps[:].rearrange("p a t -> p (a t)"), AF.Exp)
                        nc.tensor.matmul(lps[:], ones2[:], pm2[:],
                                         start=first, stop=last, perf_mode=DR)
                        nc.tensor.matmul(ops[:], v_my[:, kb0:kb0 + 2, :], pm2[:],
                                         start=first, stop=last, perf_mode=DR)
                    else:           # leftover single (diagonal) block
                        sps = a_ps.tile([P, 2, TOK], FP32, name="sps")
                        nc.tensor.matmul(sps[:, 0, :], k_my[:, qb, :], qv,
                                         start=True, stop=True)
                        pmd = a_sb.tile([P, TOK], BF16, name="pmd")
                        nc.scalar.activation(pmd[:], sps[:, 0, :], AF.Exp)
                        pmf = a_sb.tile([P, TOK], FP8, name="pmf")
                        nc.vector.tensor_mul(pmf[:], pmd[:], tril2[:])
                        nc.tensor.matmul(lps[:], ones1[:], pmf[:],
                                         start=first, stop=True)
                        nc.tensor.matmul(ops[:], v_my[:, qb, :], pmf[:],
                                         start=first, stop=True)
                lsb = a_sb.tile([1, TOK], FP32, name="lsb")
                nc.scalar.activation(lsb[:], lps[0:1, :], AF.Copy)
                bca = bc_ps.tile([P, TOK], FP32, name="bca")
                nc.tensor.matmul(bca[:], ones_f[0:1, :], lsb[:], start=True, stop=True)
                linv = a_sb.tile([P, TOK], FP32, name="linv")
                nc.vector.reciprocal_approx_fast(linv[:], bca[:])
                osb = a_sb.tile([P, TOK], FP8, name="osb")
                nc.vector.tensor_mul(osb[:], ops[:], linv[:])
                dst = a2a_lo_in if qb < 8 else a2a_hi_in
                nc.sync.dma_start(
                    dst[r_dst][:],
                    osb[:].rearrange("p (h t) -> p h t", h=2))
                if qb == 7:
                    nc.gpsimd.collective_compute(
                        "AllToAll", ALU.bypass, ins=[a2a_lo_in[:]],
                        outs=[a2a_lo_out[:]], replica_groups=rg_all)
            nc.gpsimd.collective_compute(
                "AllToAll", ALU.bypass, ins=[a2a_hi_in[:]],
                outs=[a2a_hi_out[:]], replica_groups=rg_all)
        qkvpool.release()

        # ===== phase 3: o_proj + residual + ln2 (token halves) + pair-AG =====
        with tc.tile_pool(name="wo_pool", bufs=3) as wop, \
             tc.tile_pool(name="wo_res", bufs=1) as wores, \
             tc.tile_pool(name="p5ps", bufs=2, space="PSUM") as p5ps, \
             tc.tile_pool(name="p5sb", bufs=3) as p5sb:
            nc.sync.dma_start(xo[:], xT_own[:])
            wo_all = wores.tile([P, HT, HT, P], FP8)
            for f in range(HT):
                nc.scalar.dma_start(wo_all[:, f, :, :], wo_in[f])
            for j in range(NC):
                nc.sync.dma_start(o_my[:, 2 * j:2 * j + 2, 0:P], a2a_lo_out[j])
            ssq2 = p5ps.tile([16, TOK], FP32, name="ssq2")
            for half in range(2):
                csl = slice(half * P, (half + 1) * P)
                if half == 1:
                    for j in range(NC):
                        nc.sync.dma_start(o_my[:, 2 * j:2 * j + 2, P:TOK],
                                          a2a_hi_out[j])
                for f in range(HT):
                    ps = p5ps.tile([P, P], FP32, name="ops5")
                    for kt in range(HT):
                        nc.tensor.matmul(ps[:], wo_all[:, f, kt, :], o_my[:, kt, csl],
                                         start=(kt == 0), stop=(kt == HT - 1))
                    nc.vector.scalar_tensor_tensor(
                        x_mid[:, f, csl], ps[:], ao[:, f:f + 1],
                        xo[:, f, csl], ALU.mult, ALU.add)
                for b in range(HT2):
                    sq2 = p5sb.tile([P, 2, P], FP8, name="sq2h")
                    for j in range(2):
                        nc.vector.tensor_mul(sq2[:, j, :],
                                             x_mid[:, 2 * b + j, csl],
                                             x_mid[:, 2 * b + j, csl])
                    nc.tensor.matmul(ssq2[:, csl], invh2[:], sq2[:],
                                     start=(b == 0), stop=(b == HT2 - 1),
                                     perf_mode=DR)
            ssb2 = p5sb.tile([1, TOK], FP32, name="ssb2")
            nc.scalar.activation(ssb2[:], ssq2[0:1, :], AF.Copy)
            msb2 = p5ps.tile([P, TOK], FP32, name="msb2")
            nc.tensor.matmul(msb2[:], ones_f[0:1, :], ssb2[:],
                             start=True, stop=True)
            rec2 = p5sb.tile([P, TOK], FP32, name="rec2")
            nc.vector.reciprocal_approx_fast(rec2[:], msb2[:])
            rsq2 = p5sb.tile([P, TOK], FP32, name="rsq2")
            nc.scalar.activation(rsq2[:], rec2[:], AF.Sqrt, scale=4.0)
            h2h = p5sb.tile([P, HT, TOK], FP8, name="h2h", tag="h2h")
            for kt in range(HT):
                nc.vector.tensor_mul(h2h[:, kt, :], x_mid[:, kt, :], rsq2[:])
            nc.sync.dma_start(agx_in[:], h2h[:])
            nc.gpsimd.collective_compute(
                "AllGather", ALU.bypass, ins=[agx_in[:]],
                outs=[agx_out[:]], replica_groups=rg_pair)
            nc.sync.dma_start(xmidT[:], x_mid[:])
        omypool.release()
        xopool.release()

        # ========== phase 4: MLP (pair TP-2 over inter) + chunked RS ==========
        with tc.tile_pool(name="h2c_pool", bufs=1) as h2cp, \
             tc.tile_pool(name="m_pool", bufs=1) as mp, \
             tc.tile_pool(name="wgu_pool", bufs=2) as wgup, \
             tc.tile_pool(name="wd_pool", bufs=3) as wdp, \
             tc.tile_pool(name="p7ps", bufs=2, space="PSUM") as p7ps, \
             tc.tile_pool(name="p7dps", bufs=2, space="PSUM") as p7dps, \
             tc.tile_pool(name="p7sb", bufs=4) as p7sb:
            # pair token order: [2j's 256 | (2j+1)'s 256]
            h2c = h2cp.tile([P, HT, PTOK], FP8)
            for r in range(2):
                for kh in range(2):
                    eng = nc.sync if (2 * r + kh) % 2 == 0 else nc.scalar
                    eng.dma_start(h2c[:, 8 * kh:8 * (kh + 1), r * TOK:(r + 1) * TOK],
                                  agx_out[r][:, 8 * kh:8 * (kh + 1), :])
            m_all = mp.tile([P, IT, PTOK], FP8)
            for f in range(IT):
                if f % 8 == 0:
                    wtg8 = wgup.tile([P, 8, HT2, 2, P], FP8, name="wtg8")
                    nc.sync.dma_start(wtg8[:], wg_in[f // 8])
                    wtu8 = wgup.tile([P, 8, HT2, 2, P], FP8, name="wtu8")
                    nc.scalar.dma_start(wtu8[:], wu_in[f // 8])
                gps = p7ps.tile([P, PTOK], FP32, name="gps")
                for b in range(HT2):
                    nc.tensor.matmul(gps[:], wtg8[:, f % 8, b, :, :],
                                     h2c[:, 2 * b:2 * b + 2, :],
                                     start=(b == 0), stop=(b == HT2 - 1),
                                     perf_mode=DR)
                ups = p7ps.tile([P, PTOK], FP32, name="ups")
                for b in range(HT2):
                    nc.tensor.matmul(ups[:], wtu8[:, f % 8, b, :, :],
                                     h2c[:, 2 * b:2 * b + 2, :],
                                     start=(b == 0), stop=(b == HT2 - 1),
                                     perf_mode=DR)
                gr = p7sb.tile([P, PTOK], BF16, name="gr")
                nc.vector.tensor_scalar(gr[:], gps[:], ag[:, f:f + 1], 0.0,
                                        ALU.mult, ALU.max)
                g2 = p7sb.tile([P, PTOK], BF16, name="g2")
                nc.gpsimd.tensor_mul(g2[:], gr[:], gr[:])
                nc.vector.scalar_tensor_tensor(m_all[:, f, :], ups[:],
                                               au[:, f:f + 1], g2[:],
                                               ALU.mult, ALU.mult)
            # down proj (bf16) + chunked pair-RS (4 chunks of 4 f-tiles)
            for c in range(4):
                rs_in = nc.dram_tensor(f"rs_in_{c}", [4 * P, PTOK], BF16)
                rs_iv = rs_in[:].rearrange("(f p) t -> f p t", p=P)
                rs_out = nc.dram_tensor(f"rs_out_{c}", [2 * P, PTOK], BF16)
                for fi in range(4):
                    fo = 4 * c + fi
                    wtd = wdp.tile([P, IT // 2, 2, P], FP8, name="wtd")
                    nc.sync.dma_start(wtd[:], wd_in[fo])
                    dps = p7dps.tile([P, PTOK], FP32, name="dps")
                    for b in range(IT // 2):
                        nc.tensor.matmul(dps[:], wtd[:, b, :, :],
                                         m_all[:, 2 * b:2 * b + 2, :],
                                         start=(b == 0), stop=(b == IT // 2 - 1),
                                         perf_mode=DR)
                    dn = p7sb.tile([P, PTOK], BF16, name="dn")
                    nc.scalar.activation(dn[:], dps[:], AF.Copy,
                                         scale=ad[:, fo:fo + 1])
                    nc.sync.dma_start(rs_iv[fo % 4], dn[:])
                nc.gpsimd.collective_compute(
                    "ReduceScatter", ALU.add, ins=[rs_in[:]],
                    outs=[rs_out[:]], replica_groups=rg_pair)
                nc.sync.dma_start(outD[c], rs_out[:])
            _ = wop
        midpool.release()
        const.release()

    nc.finalize()
    return nc


def _ternary(w, fold_row=None):
    """Quantize [O, Hin] fp32 -> (ternary fp32 {-1,0,1}, absmean [O])."""
    w = np.asarray(w, dtype=np.float32)
    am = np.mean(np.abs(w), axis=1)
    t = np.sign(w) * (np.abs(w) > ALPHA * am[:, None]).astype(np.float32)
    if fold_row is not None:
        t = t * fold_row[None, :]
    return t, am


def _wlhsT(tern, n_f):
    """ternary [O, Hin] -> bf16 lhsT layout [f, p, kt, c]."""
    o, hin = tern.shape
    kt = hin // P
    assert n_f * P == o
    wT = np.ascontiguousarray(tern.T)  # [Hin, O]
    return np.ascontiguousarray(
        wT.reshape(kt, P, n_f, P).transpose(2, 1, 0, 3)).astype(BF)


def _wlhsT_dr(tern, n_f):
    """ternary [O, Hin] -> fp8 DoubleRow lhsT layout [p, f, b, i, m]:
    w[p, f, b, i, m] = ternT[128*(2b+i)+p, 128*f+m]."""
    o, hin = tern.shape
    b2 = hin // (2 * P)
    assert n_f * P == o
    wT = np.ascontiguousarray(tern.T)  # [Hin, O]
    return np.ascontiguousarray(
        wT.reshape(b2, 2, P, n_f, P).transpose(2, 3, 0, 1, 4)).astype(F8)


def _wd_layout(td_slice):
    """[H, I_loc] -> fp8 DoubleRow [fo, p, b, i, m]: wd[fo, p, b, i, m] =
    td_slice[128*fo+m, 128*(2b+i)+p]."""
    hin, iloc = td_slice.shape
    assert hin == H and iloc == I_LOC
    wT = np.ascontiguousarray(td_slice.T)  # [I_loc, H]
    return np.ascontiguousarray(
        wT.reshape(IT // 2, 2, P, HT, P).transpose(3, 2, 0, 1, 4)).astype(F8)


def _scale_tiles(a):
    """[O] -> [P, O//P] with column f = features f*128..f*128+127."""
    return np.ascontiguousarray(a.reshape(-1, P).T).astype(np.float32)


def _pcol(x2d):
    """[K, T] -> [P, K//P, T] (partition-major for direct DMA)."""
    k, t = x2d.shape
    return np.ascontiguousarray(
        x2d.reshape(k // P, P, t).transpose(1, 0, 2)).astype(np.float32)


def kernel(x, cos, sin, wq, wk, wv, wo, wg, wu, wd, ln1_w, ln2_w):
    x = np.asarray(x, dtype=np.float32)
    b, s, hdim = x.shape
    assert (b, s, hdim) == (1, S, H)

    if "nc" not in _CACHE:
        _CACHE["nc"] = _build_program()
    nc = _CACHE["nc"]

    ln1 = np.asarray(ln1_w, dtype=np.float32)
    ln2 = np.asarray(ln2_w, dtype=np.float32)

    tq, amq = _ternary(wq, fold_row=ln1)
    tk, amk = _ternary(wk, fold_row=ln1)
    tv, amv = _ternary(wv, fold_row=ln1)
    to, amo = _ternary(wo)
    tg, amg = _ternary(wg, fold_row=ln2)
    tu, amu = _ternary(wu, fold_row=ln2)
    td, amd = _ternary(wd)

    wq_h = _wlhsT_dr(tq, NH)          # [P, 16, 8, 2, P]
    wk_h = _wlhsT_dr(tk, NKV)         # [P, 4, 8, 2, P]
    wv_h = _wlhsT_dr(tv, NKV)
    wo_h = _wlhsT(to, HT).astype(F8)  # [16, P, 16, P] fp8

    aq_h = _scale_tiles(amq / np.sqrt(np.float32(D)))
    ak_h = _scale_tiles(amk)
    av_h = _scale_tiles(amv)
    ao_h = _scale_tiles(amo)
    ag_h = _scale_tiles(amg)          # [P, 64]
    au_h = _scale_tiles(amu)
    ad_h = _scale_tiles(amd)          # [P, 16]

    x2 = x[0]
    xT = np.ascontiguousarray(x2.T)
    xT_f = _pcol(xT)
    cosT = np.ascontiguousarray(np.asarray(cos, np.float32)[0, 0].T).astype(BF)
    sinT = np.ascontiguousarray(np.asarray(sin, np.float32)[0, 0].T).astype(BF)

    R = np.zeros((P, P), np.float32)
    for m in range(64):
        R[m, m + 64] = -1.0
        R[m + 64, m] = 1.0
    rT_h = np.ascontiguousarray(R.T).astype(BF)
    ones_f = np.ones((P, P), np.float32)
    ones_b = np.ones((P, 1), np.float32).astype(BF)
    invh_b = np.full((P, 1), 1.0 / H, np.float32).astype(BF)
    ones2_h = np.ones((P, 2, 16), np.float32).astype(F8)
    invh2_h = np.full((P, 2, 16), 2.0 ** -9, np.float32).astype(F8)
    ones1_h = np.ones((P, 16), np.float32).astype(F8)
    triu = np.triu(np.ones((P, P), np.float32))
    tril2_h = np.ascontiguousarray(np.concatenate([triu, triu], axis=1)).astype(BF)
    iden_h = np.eye(P, dtype=np.float32).astype(BF)

    in_maps = []
    for i in range(NC):
        blo, bhi = i, 15 - i
        own_cols = np.r_[blo * P:(blo + 1) * P, bhi * P:(bhi + 1) * P]
        kvh = i // 2
        par = i % 2
        isl = slice(par * IT, (par + 1) * IT)       # inter tile slice (TP-2)
        irow = slice(par * I_LOC, (par + 1) * I_LOC)
        in_maps.append({
            "xT_f": xT_f.astype(F8),
            "xT_own": _pcol(xT[:, own_cols]),
            "cos_f": cosT, "sin_f": sinT,
            "wq": np.ascontiguousarray(wq_h[:, 2 * i:2 * i + 2]),
            "wk": np.ascontiguousarray(wk_h[:, kvh]),
            "wv": np.ascontiguousarray(wv_h[:, kvh]),
            "wo": wo_h,
            "wg": np.ascontiguousarray(
                _wlhsT_dr(tg[irow], IT).reshape(P, 4, 8, HT2, 2, P)
                .transpose(1, 0, 2, 3, 4, 5)),
            "wu": np.ascontiguousarray(
                _wlhsT_dr(tu[irow], IT).reshape(P, 4, 8, HT2, 2, P)
                .transpose(1, 0, 2, 3, 4, 5)),
            "wd": _wd_layout(td[:, irow]),
            "aq": np.ascontiguousarray(aq_h[:, 2 * i:2 * i + 2]),
            "ak": np.ascontiguousarray(ak_h[:, kvh:kvh + 1]),
            "av": np.ascontiguousarray(av_h[:, kvh:kvh + 1]),
            "ao": ao_h,
            "ag": np.ascontiguousarray(ag_h[:, isl]),
            "au": np.ascontiguousarray(au_h[:, isl]),
            "ad": ad_h,
            "rT": rT_h, "tril2": tril2_h, "iden": iden_h,
            "iden8": iden_h.astype(F8),
            "ones_f": ones_f, "ones_b": ones_b, "invh_b": invh_b,
            "ones2": ones2_h, "ones1": ones1_h, "invh2": invh2_h,
        })

    res = run_bass_kernel_spmd(nc, in_maps, list(range(NC)))
    _CACHE["last_result"] = res

    # ---- host-side unshard: xmid residual + pair-RS output assembly ----
    out_T = np.zeros((H, S), np.float64)
    for i in range(NC):
        blo, bhi = i, 15 - i
        xm = res.results[i]["xmidT"].astype(np.float64)      # [P, HT, 256]
        xm = xm.transpose(1, 0, 2).reshape(H, TOK)
        out_T[:, blo * P:(blo + 1) * P] += xm[:, 0:P]
        out_T[:, bhi * P:(bhi + 1) * P] += xm[:, P:TOK]
    for j in range(NC // 2):
        # pair token order: [core 2j's 256 | core 2j+1's 256]
        tok_cols = np.r_[(2 * j) * P:(2 * j + 1) * P,
                         (15 - 2 * j) * P:(16 - 2 * j) * P,
                         (2 * j + 1) * P:(2 * j + 2) * P,
                         (14 - 2 * j) * P:(15 - 2 * j) * P]
        for par in range(2):
            od = res.results[2 * j + par]["outD"].astype(np.float64)  # [4,256,512]
            for c in range(4):
                rows = slice(512 * c + 256 * par, 512 * c + 256 * (par + 1))
                out_T[rows][:, tok_cols] += od[c]
    return np.ascontiguousarray(out_T.T).reshape(1, S, H).astype(np.float32)


if __name__ == "__main__":
    nc = _build_program()
    print("build OK; instructions:",
          sum(len(b.instructions) for f in nc.m.functions for b in f.blocks))
